# revision 43
# baseline (speedup 1.0000x reference)
"""Neural CDE (RK4, 10 steps) Trainium2 Bass/Tile kernel — v3.

Data-parallel over batch: B=1024 split as 128 per core across 8 NeuronCores.
Weights replicated; no collectives.

Design (vs v1 baseline at ~1.41-1.68ms, now ~1.18ms):
  * W2 matmul in fp8e4 DoubleRow perf mode: K=256 contracted in a single
    instruction per 256-col window at 0.5 cyc/col — h quantizes for free
    via the tanh ACT output dtype; W2 host-quantized with a global scale
    folded into the per-stage dX multipliers.  rel_err ~1.6e-2 (gate 2e-2).
  * F streams through PSUM in 16 chunks of 1024 (3-deep PSUM ring).
    ACT plain-copies each chunk to SBUF bf16 (1 elem/cyc/lane);
    DVE multiplies by broadcast dX at 2x-packed rate.
  * Segment reduction over c as a 2x-packed bf16 pairwise add-tree on
    DVE (tensor_reduce has no DVE perf mode and runs 1x), batched into
    4-chunk L1 pieces feeding per-h'-half deep levels.
  * The stage is split into h'-halves: half-0's tree, z-update,
    PE transpose and W1 k-chunk all overlap half-1's crossing, keeping
    the serial RK4 stage tail short.
  * b2 term enters as bc = dX @ b2r.T (PE, precomputed per stage) added
    at the tree root; out rows are projected from the bf16 zT halves.
"""

import sys
import numpy as np

for _p in ("/opt/trn_rl_repo",):
    if _p not in sys.path:
        sys.path.insert(0, _p)

import ml_dtypes
from contextlib import ExitStack

import concourse.bass as bass
import concourse.bacc as bacc
import concourse.mybir as mybir
import concourse.tile as tile
from concourse.masks import make_identity
from concourse.bass_utils import run_bass_kernel_spmd

B, T, C, H = 1024, 11, 64, 256
NCORES = 8
BS = B // NCORES          # 128
HC = H * C                # 16384

# ---- tuning knobs ------------------------------------------------------
# dtype per RK4 slot (1..4): "fp8" or "bf16"
SLOT_DT = {1: "fp8", 2: "fp8", 3: "fp8", 4: "fp8"}
N_FUSED_DVE = 0           # chunks consumed by DVE directly from PSUM (fused mul)
N_GPS_MUL = 0             # chunks whose bf16 multiply runs on GpSimd

f32 = np.float32
bf16 = ml_dtypes.bfloat16
fp8 = ml_dtypes.float8_e4m3   # TRN float8e4: max ±240
FP32 = mybir.dt.float32
BF16 = mybir.dt.bfloat16
FP8 = mybir.dt.float8e4
AO = mybir.AluOpType
AF = mybir.ActivationFunctionType
AX = mybir.AxisListType
DR = mybir.MatmulPerfMode.DoubleRow

NEED_FP8 = "fp8" in SLOT_DT.values()
NEED_BF16 = "bf16" in SLOT_DT.values()

CHUNK = 1024                  # PSUM chunk cols (16 h'-rows of 64 c)
NCHUNK = HC // CHUNK          # 16 chunks; chunks [0,8) = h'-half 0


def _stage_consts(t_span: np.ndarray):
    """Host-side f32 scalar constants mimicking the reference's fp32 ops."""
    t = np.asarray(t_span, dtype=f32)
    cs = []
    for i in range(T - 1):
        t0 = t[i]
        dt = f32(t[i + 1] - t0)
        tm = f32(t0 + f32(f32(0.5) * dt))
        idx_m = int(np.clip(np.searchsorted(t, tm, side="right") - 1, 0, T - 2))
        fm = f32(tm - t[idx_m])
        cs.append((float(dt), idx_m, float(fm)))
    fr_last = f32(t[T - 1] - t[T - 2])
    return cs, float(fr_last)


def _build_program(t_span: np.ndarray, w2_scale: float):
    cs, fr_last = _stage_consts(t_span)
    s_inv = float(1.0 / w2_scale)

    nc = bacc.Bacc("TRN2", target_bir_lowering=False, debug=False,
                   enable_asserts=False, num_devices=NCORES)

    coeffs_d = nc.dram_tensor("coeffs", [BS, T - 1, 4 * C], FP32, kind="ExternalInput").ap()
    w1_d = nc.dram_tensor("w1", [H, H], BF16, kind="ExternalInput").ap()
    if NEED_FP8:
        w2f8_d = nc.dram_tensor("w2f8", [128, 2 * HC], FP8, kind="ExternalInput").ap()
    if NEED_BF16:
        w2b16_d = nc.dram_tensor("w2b16", [128, 2 * HC], BF16, kind="ExternalInput").ap()
    b1_d = nc.dram_tensor("b1", [H], FP32, kind="ExternalInput").ap()
    b2rt_d = nc.dram_tensor("b2rt", [C, H], BF16, kind="ExternalInput").ap()
    winit_d = nc.dram_tensor("winit", [C, H], BF16, kind="ExternalInput").ap()
    wout_d = nc.dram_tensor("wout", [H, C], BF16, kind="ExternalInput").ap()
    binit_d = nc.dram_tensor("binit", [1, H], FP32, kind="ExternalInput").ap()
    bout_d = nc.dram_tensor("bout", [1, C], FP32, kind="ExternalInput").ap()
    out_d = nc.dram_tensor("out", [BS, T * C], FP32, kind="ExternalOutput").ap()

    with tile.TileContext(nc) as tc, ExitStack() as ctx:
        const = ctx.enter_context(tc.tile_pool(name="const", bufs=1))
        spool = ctx.enter_context(tc.tile_pool(name="stage", bufs=2))
        zpool = ctx.enter_context(tc.tile_pool(name="z", bufs=3))
        kbpool = ctx.enter_context(tc.tile_pool(name="kb", bufs=5))
        gpool = ctx.enter_context(tc.tile_pool(name="gsb", bufs=3))
        cpool = ctx.enter_context(tc.tile_pool(name="craw", bufs=3))
        tpool = ctx.enter_context(tc.tile_pool(name="tr", bufs=2))
        pp = ctx.enter_context(tc.tile_pool(name="psmm", bufs=1, space="PSUM"))
        zt_pool = ctx.enter_context(tc.tile_pool(name="pszt", bufs=1, space="PSUM"))
        fp = ctx.enter_context(tc.tile_pool(name="psfp", bufs=3, space="PSUM"))

        # ---- resident tensors -------------------------------------------
        coeffs_sb = const.tile([BS, (T - 1) * 4 * C], FP32, tag="coeffs")
        w1_sb = const.tile([128, 2 * H], BF16, tag="w1")
        if NEED_FP8:
            w2f8_sb = const.tile([128, 2 * HC], FP8, tag="w2f8")
        if NEED_BF16:
            w2b16_sb = const.tile([128, 2 * HC], BF16, tag="w2b16")
        b1_sb = const.tile([128, 2], FP32, tag="b1")
        b2rt_sb = const.tile([C, H], BF16, tag="b2rt")
        winit_sb = const.tile([C, H], BF16, tag="winit")
        wout_sb = const.tile([128, 2 * C], BF16, tag="wout")
        binit_sb = const.tile([1, H], FP32, tag="binit")
        bout_sb = const.tile([1, C], FP32, tag="bout")
        ones1_sb = const.tile([1, 128], FP32, tag="ones1")
        ident = const.tile([128, 128], FP32, tag="ident")
        binit_rep = const.tile([128, H], FP32, tag="binit_rep")
        bout_rep = const.tile([128, C], FP32, tag="bout_rep")
        dxm_sb = const.tile([128, 11 * C], FP32, tag="dxm")      # 10 mids + last-end
        dxT_sb = const.tile([C, 21 * 128], BF16, tag="dxT")
        # per-stage DVE broadcast multipliers (fp8 variant folds 1/s)
        dxb8_sb = const.tile([128, 21 * C], BF16, tag="dxb8")
        dxb16_sb = const.tile([128, 21 * C], BF16, tag="dxb16")
        bc_all = const.tile([128, 21 * H], BF16, tag="bc_all")   # dX @ b2r.T per stage
        out_sb = const.tile([BS, T * C], FP32, tag="out_sb")

        nc.sync.dma_start(out=coeffs_sb[:], in_=coeffs_d.rearrange("p i j -> p (i j)"))
        nc.sync.dma_start(out=w1_sb.rearrange("p (k h) -> p k h", k=2),
                          in_=w1_d.rearrange("(k p) h -> p k h", p=128))
        if NEED_FP8:
            nc.sync.dma_start(out=w2f8_sb[:], in_=w2f8_d)
        if NEED_BF16:
            nc.sync.dma_start(out=w2b16_sb[:], in_=w2b16_d)
        nc.sync.dma_start(out=b1_sb[:], in_=b1_d.rearrange("(k p) -> p k", p=128))
        nc.sync.dma_start(out=b2rt_sb[:], in_=b2rt_d)
        nc.sync.dma_start(out=winit_sb[:], in_=winit_d)
        nc.sync.dma_start(out=wout_sb.rearrange("p (k c) -> p k c", k=2),
                          in_=wout_d.rearrange("(k p) c -> p k c", p=128))
        nc.sync.dma_start(out=binit_sb[:], in_=binit_d)
        nc.sync.dma_start(out=bout_sb[:], in_=bout_d)

        nc.vector.memset(ones1_sb[:], 1.0)
        make_identity(nc, ident[:])
        identb = const.tile([128, 128], BF16, tag="identb")
        nc.vector.tensor_copy(identb[:], ident[:])

        def cview(i, part):
            """f32 view of coeff column `part` (0=a,1=b,2=2c,3=3d) of interval i."""
            off = i * 4 * C + part * C
            return coeffs_sb[:, off:off + C]

        def dx_f32(s):
            if s < 10:
                return cview(s, 1)
            return dxm_sb[:, (s - 10) * C:(s - 9) * C]

        # ---- dX mid/end vectors (f32) -----------------------------------
        tmp_pool = ctx.enter_context(tc.tile_pool(name="tmp64", bufs=2))
        for i in range(T - 1):
            dt_i, im, fm = cs[i]
            tmp = tmp_pool.tile([128, C], FP32, tag="t64")
            nc.vector.scalar_tensor_tensor(
                out=tmp[:], in0=cview(im, 3), scalar=float(fm), in1=cview(im, 2),
                op0=AO.mult, op1=AO.add)
            nc.vector.scalar_tensor_tensor(
                out=dxm_sb[:, i * C:(i + 1) * C], in0=tmp[:], scalar=float(fm),
                in1=cview(im, 1), op0=AO.mult, op1=AO.add)
        tmp = tmp_pool.tile([128, C], FP32, tag="t64")
        nc.vector.scalar_tensor_tensor(
            out=tmp[:], in0=cview(T - 2, 3), scalar=float(fr_last), in1=cview(T - 2, 2),
            op0=AO.mult, op1=AO.add)
        nc.vector.scalar_tensor_tensor(
            out=dxm_sb[:, 10 * C:11 * C], in0=tmp[:], scalar=float(fr_last),
            in1=cview(T - 2, 1), op0=AO.mult, op1=AO.add)

        # ---- per-stage scaled dX variants --------------------------------
        for s in range(21):
            src = dx_f32(s)
            nc.scalar.mul(dxb8_sb[:, s * C:(s + 1) * C], src, float(s_inv))
            nc.scalar.copy(dxb16_sb[:, s * C:(s + 1) * C], src)

        # ---- dX transposes (bf16) + bc = dX @ b2r.T ----------------------
        for s in range(21):
            src = dx_f32(s)
            ps = pp.tile([128, H], FP32, tag="ht")
            nc.tensor.transpose(ps[0:C, 0:128], src, ident[:])
            nc.scalar.copy(dxT_sb[:, s * 128:(s + 1) * 128], ps[0:C, 0:128])
        for s in range(21):
            bc_ps = pp.tile([128, H], FP32, tag="ht")
            nc.tensor.matmul(bc_ps[:], lhsT=dxT_sb[:, s * 128:(s + 1) * 128],
                             rhs=b2rt_sb[:], start=True, stop=True)
            nc.scalar.copy(bc_all[:, s * H:(s + 1) * H], bc_ps[:])

        def bc_sb(s):
            return bc_all[:, s * H:(s + 1) * H]

        # ---- replicated biases -------------------------------------------
        ps = pp.tile([128, H], FP32, tag="ht")
        nc.tensor.matmul(ps[:, 0:H], lhsT=ones1_sb[:], rhs=binit_sb[:], start=True, stop=True)
        nc.scalar.copy(binit_rep[:], ps[:, 0:H])
        ps = pp.tile([128, H], FP32, tag="ht")
        nc.tensor.matmul(ps[:, 0:C], lhsT=ones1_sb[:], rhs=bout_sb[:], start=True, stop=True)
        nc.scalar.copy(bout_rep[:], ps[:, 0:C])

        # ---- z0 (as half pairs: fp32 z + bf16 cast zc) -------------------
        ps = pp.tile([128, H], FP32, tag="ht")
        nc.tensor.transpose(ps[0:C, 0:128], cview(0, 0), ident[:])
        x0T_sb = spool.tile([C, 128], BF16, tag="x0T")
        nc.scalar.copy(x0T_sb[:], ps[0:C, 0:128])
        ps = pp.tile([128, H], FP32, tag="ht")
        nc.tensor.matmul(ps[:, 0:H], lhsT=x0T_sb[:], rhs=winit_sb[:], start=True, stop=True)
        z = [zpool.tile([BS, 128], FP32, tag=f"z{h}", name=f"z{h}") for h in range(2)]
        zc = [zpool.tile([BS, 128], BF16, tag=f"zc{h}", name=f"zc{h}") for h in range(2)]
        for h in range(2):
            nc.vector.tensor_tensor(out=z[h][:], in0=ps[:, h * 128:(h + 1) * 128],
                                    in1=binit_rep[:, h * 128:(h + 1) * 128], op=AO.add)
            nc.vector.tensor_copy(zc[h][:], z[h][:])

        # ---- one RK4 stage ----------------------------------------------
        # zin is a pair of [128,128] half tiles; returns kb = pair of
        # [128,128] fp32 half tiles.
        def gstage(zin, s, slot, emit_out_t=None, post_half=None):
            dt8 = SLOT_DT[slot] == "fp8"
            # per-half: transpose zin-half (bf16) -> zT-half -> W1 k-chunk
            ht_ps = pp.tile([128, H], FP32, tag="ht")
            ot_ps = (fp.tile([128, CHUNK], FP32, tag="fp", name="ot_ps")
                     if emit_out_t is not None else None)
            ztpair = zt_pool.tile([128, 256], BF16, tag="ztb", name="ztpair")
            for hh in range(2):
                nc.tensor.transpose(ztpair[:, hh * 128:(hh + 1) * 128],
                                    zin[hh][:], identb[:])
            # per-half zT copies: half-0's copy runs mid-stage, only
            # half-1's sits on the inter-stage tail
            zTbs = []
            for hh in range(2):
                zTbh = spool.tile([128, 128], BF16, tag=f"zTb{hh}", name=f"zTb{hh}")
                nc.scalar.copy(zTbh[:], ztpair[:, hh * 128:(hh + 1) * 128])
                zTbs.append(zTbh[:])
            hT = spool.tile([128, H], FP8 if dt8 else BF16,
                            tag="hT8" if dt8 else "hT16", name="hT")
            hT2 = hT.rearrange("p (k m) -> p k m", k=2)
            # tanh(hck) issues as soon as its W1 region closes, overlapping
            # the other region's matmuls (ACT op between PE groups is safe)
            for hck in range(2):
                for hh in range(2):
                    nc.tensor.matmul(
                        ht_ps[:, hck * 128:(hck + 1) * 128],
                        lhsT=w1_sb[:, hh * H + hck * 128: hh * H + (hck + 1) * 128],
                        rhs=zTbs[hh],
                        start=(hh == 0), stop=(hh == 1))
                nc.scalar.activation(hT[:, hck * 128:(hck + 1) * 128],
                                     ht_ps[:, hck * 128:(hck + 1) * 128],
                                     AF.Tanh, bias=b1_sb[:, hck:hck + 1], scale=1.0)
            if ot_ps is not None:
                for hh in range(2):
                    nc.tensor.matmul(ot_ps[:, 0:C], lhsT=zTbs[hh],
                                     rhs=wout_sb[:, hh * C:(hh + 1) * C],
                                     start=(hh == 0), stop=(hh == 1))
                nc.vector.tensor_tensor(out=out_sb[:, emit_out_t * C:(emit_out_t + 1) * C],
                                        in0=ot_ps[:, 0:C], in1=bout_rep[:], op=AO.add)

            if NEED_FP8 and dt8:
                w2v = w2f8_sb.rearrange("p (k n) -> p k n", k=2)
            else:
                w2v = w2b16_sb.rearrange("p (k n) -> p k n", k=2)
            dxb = (dxb8_sb if dt8 else dxb16_sb)[:, s * C:(s + 1) * C]
            dxbc = dxb.rearrange("p (r c) -> p r c", r=1)

            def do_mm(fps, col0, ncols):
                if dt8:
                    for j in range(ncols // 256):
                        nc.tensor.matmul(
                            fps[:, j * 256:(j + 1) * 256],
                            lhsT=hT2, rhs=w2v[:, :, col0 + j * 256: col0 + (j + 1) * 256],
                            start=True, stop=True, perf_mode=DR,
                            skip_group_check=True)
                else:
                    for w in range(ncols // 512):
                        for kc in range(2):
                            nc.tensor.matmul(
                                fps[:, w * 512:(w + 1) * 512],
                                lhsT=hT[:, kc * 128:(kc + 1) * 128],
                                rhs=w2v[:, kc, col0 + w * 512: col0 + (w + 1) * 512],
                                start=(kc == 0), stop=(kc == 1),
                                skip_group_check=True)

            # fused chunks: DVE multiplies straight from PSUM (skips ACT);
            # placed at the end of each half where the ACT queue would
            # otherwise gate the reduce tail
            fused = set()
            gps_mul = set()
            if N_GPS_MUL:
                step = max(1, (NCHUNK - N_FUSED_DVE) // N_GPS_MUL)
                gps_mul = set(range(1, NCHUNK - N_FUSED_DVE, step))

            rows = CHUNK // C              # 16 h'-rows per chunk

            def consume(q, ghalf, qq, solo=False):
                # PE chunk -> ACT copy (own ring tile) -> DVE 2x multiply
                fps = fp.tile([128, CHUNK], FP32, tag="fp", name="fps")
                do_mm(fps, q * CHUNK, CHUNK)
                craw = cpool.tile([128, CHUNK], BF16, tag="craw", name="craw")
                nc.scalar.copy(craw[:], fps[:])
                nc.vector.tensor_tensor(
                    out=ghalf[:, qq * CHUNK:(qq + 1) * CHUNK].rearrange(
                        "p (a c) -> p a c", c=C),
                    in0=craw.rearrange("p (a c) -> p a c", c=C),
                    in1=dxbc.broadcast_to([128, rows, C]), op=AO.mult)

            def tree(t1, nrows):
                # [p, nrows, 32] -> [p, nrows] via halving adds
                cur, width = t1, 32
                while width > 2:
                    nw = width // 2
                    vv = cur.rearrange("p (a two c) -> p a two c", two=2, c=nw)
                    dst = tpool.tile([BS, nrows * nw], BF16, tag=f"tl{nw}",
                                     name=f"tl{nw}")
                    nc.vector.tensor_tensor(
                        out=dst.rearrange("p (a c) -> p a c", c=nw),
                        in0=vv[:, :, 0, :], in1=vv[:, :, 1, :], op=AO.add)
                    cur, width = dst, nw
                vv = cur.rearrange("p (a two) -> p a two", two=2)
                t6 = tpool.tile([BS, nrows], BF16, tag="t6", name="t6")
                nc.vector.tensor_tensor(out=t6.rearrange("p (a o) -> p a o", o=1),
                                        in0=vv[:, :, 0:1], in1=vv[:, :, 1:2],
                                        op=AO.add)
                return t6

            kb_halves = []

            def l1p(t1x, ghalf, lo, hi):
                piece = ghalf[:, lo * CHUNK:hi * CHUNK]
                gv = piece.rearrange("p (a two c) -> p a two c", two=2, c=32)
                nc.vector.tensor_tensor(
                    out=t1x.rearrange("p (a c) -> p a c", c=32)[
                        :, lo * rows:hi * rows],
                    in0=gv[:, :, 0, :], in1=gv[:, :, 1, :], op=AO.add)

            # ---- half 0: chunks 0-7, quad muls + [4,4] L1 ---------------
            t1 = tpool.tile([BS, 128 * 32], BF16, tag="t1")
            ghalf = gpool.tile([128, 8 * CHUNK], BF16, tag="g")
            for qq in range(8):
                consume(qq, ghalf, qq)
                if qq % 4 == 3:
                    l1p(t1, ghalf, qq - 3, qq + 1)
            t6 = tree(t1, 128)
            kb0 = kbpool.tile([BS, 128], FP32, tag="kb0", name="kb0")
            nc.vector.tensor_tensor(out=kb0[:], in0=t6[:], in1=bc_sb(s)[:, 0:128],
                                    op=AO.add)
            kb_halves.append(kb0)
            if post_half is not None:
                post_half(0, kb0)

            # ---- half 1: chunks 8-15, fine tail pieces ------------------
            t1b = tpool.tile([BS, 128 * 32], BF16, tag="t1")
            ghalf = gpool.tile([128, 8 * CHUNK], BF16, tag="g")
            for qq in range(8):
                consume(8 + qq, ghalf, qq)
                if qq == 3:
                    l1p(t1b, ghalf, 0, 4)
                elif qq == 7:
                    l1p(t1b, ghalf, 4, 8)
            t6b = tree(t1b, 128)
            kb1 = kbpool.tile([BS, 128], FP32, tag="kb1", name="kb1")
            nc.vector.tensor_tensor(out=kb1[:], in0=t6b[:],
                                    in1=bc_sb(s)[:, 128:256], op=AO.add)
            kb_halves.append(kb1)
            if post_half is not None:
                post_half(1, kb1)
            return kb_halves

        # ---- RK4 time loop ----------------------------------------------
        for i in range(T - 1):
            dt_i, im, fm = cs[i]
            hdt = float(f32(f32(0.5) * f32(dt_i)))
            dt6 = float(f32(f32(dt_i) / f32(6.0)))
            s_m = 10 + i
            s_e = (i + 1) if i < T - 2 else 20

            def mk_zs(kb_alpha, tag):
                tiles = [zpool.tile([BS, 128], BF16, tag=f"{tag}{h}", name=f"{tag}{h}") for h in range(2)]

                def post(hh, kbh):
                    nc.vector.scalar_tensor_tensor(
                        out=tiles[hh][:], in0=kbh[:], scalar=kb_alpha,
                        in1=z[hh][:], op0=AO.mult, op1=AO.add)
                return tiles, post

            zs1, post1 = mk_zs(hdt, "zs1_")
            kb1 = gstage(zc, i, 1, emit_out_t=i, post_half=post1)

            zs2, post2 = mk_zs(hdt, "zs2_")
            kb2 = gstage(zs1, s_m, 2, post_half=post2)

            zs3, post3 = mk_zs(float(dt_i), "zs3_")
            kb3 = gstage(zs2, s_m, 3, post_half=post3)

            # partial RK4 combine: lands in DVE's idle window at k4 start
            acc2 = [kbpool.tile([BS, 128], FP32, tag=f"ac2{h}", name=f"ac2{h}") for h in range(2)]
            for h in range(2):
                acc = kbpool.tile([BS, 128], FP32, tag=f"acc{h}", name=f"acc{h}")
                nc.vector.scalar_tensor_tensor(out=acc[:], in0=kb2[h][:], scalar=2.0,
                                               in1=kb1[h][:], op0=AO.mult, op1=AO.add)
                nc.vector.scalar_tensor_tensor(out=acc2[h][:], in0=kb3[h][:], scalar=2.0,
                                               in1=acc[:], op0=AO.mult, op1=AO.add)

            znew = [zpool.tile([BS, 128], FP32, tag=f"z{h}", name=f"zn{h}") for h in range(2)]
            zcn = [zpool.tile([BS, 128], BF16, tag=f"zc{h}", name=f"zcn{h}") for h in range(2)]

            def post4(hh, kbh):
                acc3 = kbpool.tile([BS, 128], FP32, tag=f"ac3{hh}", name=f"ac3{hh}")
                nc.vector.tensor_tensor(out=acc3[:], in0=kbh[:], in1=acc2[hh][:],
                                        op=AO.add)
                nc.vector.scalar_tensor_tensor(out=znew[hh][:], in0=acc3[:], scalar=dt6,
                                               in1=z[hh][:], op0=AO.mult, op1=AO.add)
                nc.vector.tensor_copy(zcn[hh][:], znew[hh][:])

            gstage(zs3, s_e, 4, post_half=post4)
            z, zc = znew, zcn

        # ---- final out row (t = T-1) ------------------------------------
        ot_ps = pp.tile([128, H], FP32, tag="ht")
        for hh in range(2):
            ztp = zt_pool.tile([128, 128], BF16, tag="ztb")
            nc.tensor.transpose(ztp[:], zc[hh][:], identb[:])
            zTbh = spool.tile([128, 128], BF16, tag=f"zTb{hh}", name=f"zTb{hh}")
            nc.scalar.copy(zTbh[:], ztp[:])
            nc.tensor.matmul(ot_ps[:, 0:C], lhsT=zTbh[:],
                             rhs=wout_sb[:, hh * C:(hh + 1) * C],
                             start=(hh == 0), stop=(hh == 1))
        nc.vector.tensor_tensor(out=out_sb[:, (T - 1) * C:T * C],
                                in0=ot_ps[:, 0:C], in1=bout_rep[:], op=AO.add)

        nc.sync.dma_start(out=out_d, in_=out_sb[:])

    nc.compile()
    return nc


def _w2_cmajor(W2):
    """Fold W2 [H, H*C] (c-minor column order) rows into the DoubleRow
    [p, (ktile, n)] rhs layout."""
    wcm = np.asarray(W2, dtype=f32).reshape(H, HC)
    return wcm.reshape(2, 128, HC).transpose(1, 0, 2).reshape(128, 2 * HC)


_CACHE = {}


def _get_program(t_span: np.ndarray, w2_scale: float):
    key = (np.asarray(t_span, dtype=f32).tobytes(), float(w2_scale))
    if key not in _CACHE:
        _CACHE[key] = _build_program(t_span, w2_scale)
    return _CACHE[key]


def _w2_scale(W2):
    return float(240.0 / (1.25 * np.abs(np.asarray(W2, dtype=f32)).max()))


def _make_in_maps(inputs):
    coeffs = np.ascontiguousarray(inputs["coeffs"], dtype=f32)
    assert coeffs.shape == (B, T - 1, 4 * C)
    wcm = _w2_cmajor(inputs["W2"])
    s = _w2_scale(inputs["W2"])
    shared = {
        "w1": np.ascontiguousarray(inputs["W1"], dtype=f32).astype(bf16),
        "b1": np.ascontiguousarray(inputs["b1"], dtype=f32),
        "b2rt": np.ascontiguousarray(
            np.asarray(inputs["b2"], dtype=f32).reshape(H, C).T).astype(bf16),
        "winit": np.ascontiguousarray(inputs["W_init"], dtype=f32).astype(bf16),
        "wout": np.ascontiguousarray(inputs["W_out"], dtype=f32).astype(bf16),
        "binit": np.ascontiguousarray(inputs["b_init"], dtype=f32).reshape(1, H),
        "bout": np.ascontiguousarray(inputs["b_out"], dtype=f32).reshape(1, C),
    }
    if NEED_FP8:
        shared["w2f8"] = np.ascontiguousarray(
            np.clip(wcm * s, -240.0, 240.0)).astype(fp8)
    if NEED_BF16:
        shared["w2b16"] = np.ascontiguousarray(wcm).astype(bf16)
    in_maps = []
    for c in range(NCORES):
        m = dict(shared)
        m["coeffs"] = coeffs[c * BS:(c + 1) * BS]
        in_maps.append(m)
    return in_maps


def kernel(coeffs, t_span, W_init, b_init, W1, b1, W2, b2, W_out, b_out):
    nc = _get_program(t_span, _w2_scale(W2))
    in_maps = _make_in_maps(dict(coeffs=coeffs, W_init=W_init, b_init=b_init,
                                 W1=W1, b1=b1, W2=W2, b2=b2,
                                 W_out=W_out, b_out=b_out))
    res = run_bass_kernel_spmd(nc, in_maps, list(range(NCORES)))
    shards = [res.results[c]["out"].reshape(BS, T, C) for c in range(NCORES)]
    return np.ascontiguousarray(np.concatenate(shards, axis=0), dtype=f32)


if __name__ == "__main__":
    rng = np.random.default_rng(0)
    demo = dict(
        coeffs=(rng.standard_normal((B, T - 1, 4 * C)) * 0.5).astype(f32),
        t_span=(np.arange(T) * 0.05).astype(f32),
        W_init=(rng.standard_normal((C, H)) / 8).astype(f32),
        b_init=(rng.standard_normal((H,)) * 0.01).astype(f32),
        W1=(rng.standard_normal((H, H)) / 16).astype(f32),
        b1=(rng.standard_normal((H,)) * 0.01).astype(f32),
        W2=(rng.standard_normal((H, HC)) / 16).astype(f32),
        b2=(rng.standard_normal((HC,)) * 0.01).astype(f32),
        W_out=(rng.standard_normal((H, C)) / 16).astype(f32),
        b_out=np.zeros((C,), f32),
    )
    out = kernel(**demo)
    print("out", out.shape, out.dtype, float(np.abs(out).max()))


# revision 44
# speedup vs baseline: 1.1064x; 1.1064x over previous
"""Neural CDE (RK4, 10 steps) Trainium2 Bass/Tile kernel — v3.

Data-parallel over batch: B=1024 split as 128 per core across 8 NeuronCores.
Weights replicated; no collectives.

Design (vs v1 baseline at ~1.41-1.68ms, now ~1.18ms):
  * W2 matmul in fp8e4 DoubleRow perf mode: K=256 contracted in a single
    instruction per 256-col window at 0.5 cyc/col — h quantizes for free
    via the tanh ACT output dtype; W2 host-quantized with a global scale
    folded into the per-stage dX multipliers.  rel_err ~1.6e-2 (gate 2e-2).
  * F streams through PSUM in 16 chunks of 1024 (3-deep PSUM ring).
    ACT plain-copies each chunk to SBUF bf16 (1 elem/cyc/lane);
    DVE multiplies by broadcast dX at 2x-packed rate.
  * Segment reduction over c as a 2x-packed bf16 pairwise add-tree on
    DVE (tensor_reduce has no DVE perf mode and runs 1x), batched into
    4-chunk L1 pieces feeding per-h'-half deep levels.
  * The stage is split into h'-halves: half-0's tree, z-update,
    PE transpose and W1 k-chunk all overlap half-1's crossing, keeping
    the serial RK4 stage tail short.
  * b2 term enters as bc = dX @ b2r.T (PE, precomputed per stage) added
    at the tree root; out rows are projected from the bf16 zT halves.
"""

import sys
import numpy as np

for _p in ("/opt/trn_rl_repo",):
    if _p not in sys.path:
        sys.path.insert(0, _p)

import ml_dtypes
from contextlib import ExitStack

import concourse.bass as bass
import concourse.bacc as bacc
import concourse.mybir as mybir
import concourse.tile as tile
from concourse.masks import make_identity
from concourse.bass_utils import run_bass_kernel_spmd

B, T, C, H = 1024, 11, 64, 256
NCORES = 8
BS = B // NCORES          # 128
HC = H * C                # 16384

# ---- tuning knobs ------------------------------------------------------
# dtype per RK4 slot (1..4): "fp8" or "bf16"
SLOT_DT = {1: "fp8", 2: "fp8", 3: "fp8", 4: "fp8"}
N_FUSED_DVE = 0           # chunks consumed by DVE directly from PSUM (fused mul)
N_GPS_MUL = 0             # chunks whose bf16 multiply runs on GpSimd

f32 = np.float32
bf16 = ml_dtypes.bfloat16
fp8 = ml_dtypes.float8_e4m3   # TRN float8e4: max ±240
FP32 = mybir.dt.float32
BF16 = mybir.dt.bfloat16
FP8 = mybir.dt.float8e4
AO = mybir.AluOpType
AF = mybir.ActivationFunctionType
AX = mybir.AxisListType
DR = mybir.MatmulPerfMode.DoubleRow

NEED_FP8 = "fp8" in SLOT_DT.values()
NEED_BF16 = "bf16" in SLOT_DT.values()

CHUNK = 1024                  # PSUM chunk cols (16 h'-rows of 64 c)
NCHUNK = HC // CHUNK          # 16 chunks; chunks [0,8) = h'-half 0


def _stage_consts(t_span: np.ndarray):
    """Host-side f32 scalar constants mimicking the reference's fp32 ops."""
    t = np.asarray(t_span, dtype=f32)
    cs = []
    for i in range(T - 1):
        t0 = t[i]
        dt = f32(t[i + 1] - t0)
        tm = f32(t0 + f32(f32(0.5) * dt))
        idx_m = int(np.clip(np.searchsorted(t, tm, side="right") - 1, 0, T - 2))
        fm = f32(tm - t[idx_m])
        cs.append((float(dt), idx_m, float(fm)))
    fr_last = f32(t[T - 1] - t[T - 2])
    return cs, float(fr_last)


def _build_program(t_span: np.ndarray, w2_scale: float):
    cs, fr_last = _stage_consts(t_span)
    s_inv = float(1.0 / w2_scale)

    nc = bacc.Bacc("TRN2", target_bir_lowering=False, debug=False,
                   enable_asserts=False, num_devices=NCORES)

    coeffs_d = nc.dram_tensor("coeffs", [BS, T - 1, 4 * C], FP32, kind="ExternalInput").ap()
    w1_d = nc.dram_tensor("w1", [H, H], BF16, kind="ExternalInput").ap()
    if NEED_FP8:
        w2f8_d = nc.dram_tensor("w2f8", [128, 2 * HC], FP8, kind="ExternalInput").ap()
    if NEED_BF16:
        w2b16_d = nc.dram_tensor("w2b16", [128, 2 * HC], BF16, kind="ExternalInput").ap()
    b1_d = nc.dram_tensor("b1", [H], FP32, kind="ExternalInput").ap()
    b2rt_d = nc.dram_tensor("b2rt", [C, H], BF16, kind="ExternalInput").ap()
    winit_d = nc.dram_tensor("winit", [C, H], BF16, kind="ExternalInput").ap()
    wout_d = nc.dram_tensor("wout", [H, C], BF16, kind="ExternalInput").ap()
    binit_d = nc.dram_tensor("binit", [1, H], FP32, kind="ExternalInput").ap()
    bout_d = nc.dram_tensor("bout", [1, C], FP32, kind="ExternalInput").ap()
    out_d = nc.dram_tensor("out", [BS, T * C], FP32, kind="ExternalOutput").ap()

    with tile.TileContext(nc) as tc, ExitStack() as ctx:
        const = ctx.enter_context(tc.tile_pool(name="const", bufs=1))
        spool = ctx.enter_context(tc.tile_pool(name="stage", bufs=2))
        zpool = ctx.enter_context(tc.tile_pool(name="z", bufs=3))
        kbpool = ctx.enter_context(tc.tile_pool(name="kb", bufs=5))
        gpool = ctx.enter_context(tc.tile_pool(name="gsb", bufs=3))
        cpool = ctx.enter_context(tc.tile_pool(name="craw", bufs=3))
        tpool = ctx.enter_context(tc.tile_pool(name="tr", bufs=2))
        pp = ctx.enter_context(tc.tile_pool(name="psmm", bufs=1, space="PSUM"))
        zt_pool = ctx.enter_context(tc.tile_pool(name="pszt", bufs=1, space="PSUM"))
        fp = ctx.enter_context(tc.tile_pool(name="psfp", bufs=3, space="PSUM"))

        # ---- resident tensors -------------------------------------------
        coeffs_sb = const.tile([BS, (T - 1) * 4 * C], FP32, tag="coeffs")
        w1_sb = const.tile([128, 2 * H], BF16, tag="w1")
        if NEED_FP8:
            w2f8_sb = const.tile([128, 2 * HC], FP8, tag="w2f8")
        if NEED_BF16:
            w2b16_sb = const.tile([128, 2 * HC], BF16, tag="w2b16")
        b1_sb = const.tile([128, 2], FP32, tag="b1")
        b2rt_sb = const.tile([C, H], BF16, tag="b2rt")
        winit_sb = const.tile([C, H], BF16, tag="winit")
        wout_sb = const.tile([128, 2 * C], BF16, tag="wout")
        binit_sb = const.tile([1, H], FP32, tag="binit")
        bout_sb = const.tile([1, C], FP32, tag="bout")
        ones1_sb = const.tile([1, 128], FP32, tag="ones1")
        ident = const.tile([128, 128], FP32, tag="ident")
        binit_rep = const.tile([128, H], FP32, tag="binit_rep")
        bout_rep = const.tile([128, C], FP32, tag="bout_rep")
        dxm_sb = const.tile([128, 11 * C], FP32, tag="dxm")      # 10 mids + last-end
        dxT_sb = const.tile([C, 21 * 128], BF16, tag="dxT")
        # per-stage DVE broadcast multipliers (fp8 variant folds 1/s)
        dxb8_sb = const.tile([128, 21 * C], BF16, tag="dxb8")
        dxb16_sb = const.tile([128, 21 * C], BF16, tag="dxb16")
        bc_all = const.tile([128, 21 * H], BF16, tag="bc_all")   # dX @ b2r.T per stage
        out_sb = const.tile([BS, T * C], FP32, tag="out_sb")

        nc.sync.dma_start(out=coeffs_sb[:], in_=coeffs_d.rearrange("p i j -> p (i j)"))
        nc.sync.dma_start(out=w1_sb.rearrange("p (k h) -> p k h", k=2),
                          in_=w1_d.rearrange("(k p) h -> p k h", p=128))
        if NEED_FP8:
            nc.sync.dma_start(out=w2f8_sb[:], in_=w2f8_d)
        if NEED_BF16:
            nc.sync.dma_start(out=w2b16_sb[:], in_=w2b16_d)
        nc.sync.dma_start(out=b1_sb[:], in_=b1_d.rearrange("(k p) -> p k", p=128))
        nc.sync.dma_start(out=b2rt_sb[:], in_=b2rt_d)
        nc.sync.dma_start(out=winit_sb[:], in_=winit_d)
        nc.sync.dma_start(out=wout_sb.rearrange("p (k c) -> p k c", k=2),
                          in_=wout_d.rearrange("(k p) c -> p k c", p=128))
        nc.sync.dma_start(out=binit_sb[:], in_=binit_d)
        nc.sync.dma_start(out=bout_sb[:], in_=bout_d)

        nc.vector.memset(ones1_sb[:], 1.0)
        make_identity(nc, ident[:])
        identb = const.tile([128, 128], BF16, tag="identb")
        nc.vector.tensor_copy(identb[:], ident[:])

        def cview(i, part):
            """f32 view of coeff column `part` (0=a,1=b,2=2c,3=3d) of interval i."""
            off = i * 4 * C + part * C
            return coeffs_sb[:, off:off + C]

        def dx_f32(s):
            if s < 10:
                return cview(s, 1)
            return dxm_sb[:, (s - 10) * C:(s - 9) * C]

        # ---- dX mid/end vectors (f32) -----------------------------------
        tmp_pool = ctx.enter_context(tc.tile_pool(name="tmp64", bufs=2))
        for i in range(T - 1):
            dt_i, im, fm = cs[i]
            tmp = tmp_pool.tile([128, C], FP32, tag="t64")
            nc.vector.scalar_tensor_tensor(
                out=tmp[:], in0=cview(im, 3), scalar=float(fm), in1=cview(im, 2),
                op0=AO.mult, op1=AO.add)
            nc.vector.scalar_tensor_tensor(
                out=dxm_sb[:, i * C:(i + 1) * C], in0=tmp[:], scalar=float(fm),
                in1=cview(im, 1), op0=AO.mult, op1=AO.add)
        tmp = tmp_pool.tile([128, C], FP32, tag="t64")
        nc.vector.scalar_tensor_tensor(
            out=tmp[:], in0=cview(T - 2, 3), scalar=float(fr_last), in1=cview(T - 2, 2),
            op0=AO.mult, op1=AO.add)
        nc.vector.scalar_tensor_tensor(
            out=dxm_sb[:, 10 * C:11 * C], in0=tmp[:], scalar=float(fr_last),
            in1=cview(T - 2, 1), op0=AO.mult, op1=AO.add)

        # ---- per-stage scaled dX variants --------------------------------
        for s in range(21):
            src = dx_f32(s)
            nc.scalar.mul(dxb8_sb[:, s * C:(s + 1) * C], src, float(s_inv))
            nc.scalar.copy(dxb16_sb[:, s * C:(s + 1) * C], src)

        # ---- dX transposes (bf16) + bc = dX @ b2r.T ----------------------
        for s in range(21):
            src = dx_f32(s)
            ps = pp.tile([128, H], FP32, tag="ht")
            nc.tensor.transpose(ps[0:C, 0:128], src, ident[:])
            nc.scalar.copy(dxT_sb[:, s * 128:(s + 1) * 128], ps[0:C, 0:128])
        for s in range(21):
            bc_ps = pp.tile([128, H], FP32, tag="ht")
            nc.tensor.matmul(bc_ps[:], lhsT=dxT_sb[:, s * 128:(s + 1) * 128],
                             rhs=b2rt_sb[:], start=True, stop=True)
            nc.scalar.copy(bc_all[:, s * H:(s + 1) * H], bc_ps[:])

        def bc_sb(s):
            return bc_all[:, s * H:(s + 1) * H]

        # ---- replicated biases -------------------------------------------
        ps = pp.tile([128, H], FP32, tag="ht")
        nc.tensor.matmul(ps[:, 0:H], lhsT=ones1_sb[:], rhs=binit_sb[:], start=True, stop=True)
        nc.scalar.copy(binit_rep[:], ps[:, 0:H])
        ps = pp.tile([128, H], FP32, tag="ht")
        nc.tensor.matmul(ps[:, 0:C], lhsT=ones1_sb[:], rhs=bout_sb[:], start=True, stop=True)
        nc.scalar.copy(bout_rep[:], ps[:, 0:C])

        # ---- z0 (as half pairs: fp32 z + bf16 cast zc) -------------------
        ps = pp.tile([128, H], FP32, tag="ht")
        nc.tensor.transpose(ps[0:C, 0:128], cview(0, 0), ident[:])
        x0T_sb = spool.tile([C, 128], BF16, tag="x0T")
        nc.scalar.copy(x0T_sb[:], ps[0:C, 0:128])
        ps = pp.tile([128, H], FP32, tag="ht")
        nc.tensor.matmul(ps[:, 0:H], lhsT=x0T_sb[:], rhs=winit_sb[:], start=True, stop=True)
        z = [zpool.tile([BS, 128], FP32, tag=f"z{h}", name=f"z{h}") for h in range(2)]
        zc = [zpool.tile([BS, 128], BF16, tag=f"zc{h}", name=f"zc{h}") for h in range(2)]
        for h in range(2):
            nc.vector.tensor_tensor(out=z[h][:], in0=ps[:, h * 128:(h + 1) * 128],
                                    in1=binit_rep[:, h * 128:(h + 1) * 128], op=AO.add)
            nc.vector.tensor_copy(zc[h][:], z[h][:])

        # ---- one RK4 stage ----------------------------------------------
        # zin is a pair of [128,128] half tiles; returns kb = pair of
        # [128,128] fp32 half tiles.
        def gstage(zin, s, slot, emit_out_t=None, post_half=None):
            dt8 = SLOT_DT[slot] == "fp8"
            # per-half: transpose zin-half (bf16) -> zT-half -> W1 k-chunk
            ht_ps = pp.tile([128, H], FP32, tag="ht")
            ot_ps = (fp.tile([128, CHUNK], FP32, tag="fp", name="ot_ps")
                     if emit_out_t is not None else None)
            ztpair = zt_pool.tile([128, 256], BF16, tag="ztb", name="ztpair")
            for hh in range(2):
                nc.tensor.transpose(ztpair[:, hh * 128:(hh + 1) * 128],
                                    zin[hh][:], identb[:])
            zTb = spool.tile([128, 256], BF16, tag="zTb", name="zTb")
            nc.scalar.copy(zTb[:], ztpair[:])
            zTbs = [zTb[:, 0:128], zTb[:, 128:256]]
            for hck in range(2):
                for hh in range(2):
                    nc.tensor.matmul(
                        ht_ps[:, hck * 128:(hck + 1) * 128],
                        lhsT=w1_sb[:, hh * H + hck * 128: hh * H + (hck + 1) * 128],
                        rhs=zTbs[hh],
                        start=(hh == 0), stop=(hh == 1))
            if ot_ps is not None:
                for hh in range(2):
                    nc.tensor.matmul(ot_ps[:, 0:C], lhsT=zTbs[hh],
                                     rhs=wout_sb[:, hh * C:(hh + 1) * C],
                                     start=(hh == 0), stop=(hh == 1))
                nc.vector.tensor_tensor(out=out_sb[:, emit_out_t * C:(emit_out_t + 1) * C],
                                        in0=ot_ps[:, 0:C], in1=bout_rep[:], op=AO.add)

            hT = spool.tile([128, H], FP8 if dt8 else BF16,
                            tag="hT8" if dt8 else "hT16", name="hT")
            for hck in range(2):
                nc.scalar.activation(hT[:, hck * 128:(hck + 1) * 128],
                                     ht_ps[:, hck * 128:(hck + 1) * 128],
                                     AF.Tanh, bias=b1_sb[:, hck:hck + 1], scale=1.0)
            hT2 = hT.rearrange("p (k m) -> p k m", k=2)

            if NEED_FP8 and dt8:
                w2v = w2f8_sb.rearrange("p (k n) -> p k n", k=2)
            else:
                w2v = w2b16_sb.rearrange("p (k n) -> p k n", k=2)
            dxb = (dxb8_sb if dt8 else dxb16_sb)[:, s * C:(s + 1) * C]
            dxbc = dxb.rearrange("p (r c) -> p r c", r=1)

            def do_mm(fps, col0, ncols):
                if dt8:
                    for j in range(ncols // 256):
                        nc.tensor.matmul(
                            fps[:, j * 256:(j + 1) * 256],
                            lhsT=hT2, rhs=w2v[:, :, col0 + j * 256: col0 + (j + 1) * 256],
                            start=True, stop=True, perf_mode=DR,
                            skip_group_check=True)
                else:
                    for w in range(ncols // 512):
                        for kc in range(2):
                            nc.tensor.matmul(
                                fps[:, w * 512:(w + 1) * 512],
                                lhsT=hT[:, kc * 128:(kc + 1) * 128],
                                rhs=w2v[:, kc, col0 + w * 512: col0 + (w + 1) * 512],
                                start=(kc == 0), stop=(kc == 1),
                                skip_group_check=True)

            # fused chunks: DVE multiplies straight from PSUM (skips ACT);
            # placed at the end of each half where the ACT queue would
            # otherwise gate the reduce tail
            fused = set()
            gps_mul = set()
            if N_GPS_MUL:
                step = max(1, (NCHUNK - N_FUSED_DVE) // N_GPS_MUL)
                gps_mul = set(range(1, NCHUNK - N_FUSED_DVE, step))

            rows = CHUNK // C              # 16 h'-rows per chunk

            def consume(q, ghalf, qq, solo=False):
                # PE chunk -> ACT copy (own ring tile) -> DVE 2x multiply
                fps = fp.tile([128, CHUNK], FP32, tag="fp", name="fps")
                do_mm(fps, q * CHUNK, CHUNK)
                craw = cpool.tile([128, CHUNK], BF16, tag="craw", name="craw")
                nc.scalar.copy(craw[:], fps[:])
                nc.vector.tensor_tensor(
                    out=ghalf[:, qq * CHUNK:(qq + 1) * CHUNK].rearrange(
                        "p (a c) -> p a c", c=C),
                    in0=craw.rearrange("p (a c) -> p a c", c=C),
                    in1=dxbc.broadcast_to([128, rows, C]), op=AO.mult)

            def tree(t1, nrows):
                # [p, nrows, 32] -> [p, nrows] via halving adds
                cur, width = t1, 32
                while width > 2:
                    nw = width // 2
                    vv = cur.rearrange("p (a two c) -> p a two c", two=2, c=nw)
                    dst = tpool.tile([BS, nrows * nw], BF16, tag=f"tl{nw}",
                                     name=f"tl{nw}")
                    nc.vector.tensor_tensor(
                        out=dst.rearrange("p (a c) -> p a c", c=nw),
                        in0=vv[:, :, 0, :], in1=vv[:, :, 1, :], op=AO.add)
                    cur, width = dst, nw
                vv = cur.rearrange("p (a two) -> p a two", two=2)
                t6 = tpool.tile([BS, nrows], BF16, tag="t6", name="t6")
                nc.vector.tensor_tensor(out=t6.rearrange("p (a o) -> p a o", o=1),
                                        in0=vv[:, :, 0:1], in1=vv[:, :, 1:2],
                                        op=AO.add)
                return t6

            kb_halves = []

            def l1p(t1x, ghalf, lo, hi):
                piece = ghalf[:, lo * CHUNK:hi * CHUNK]
                gv = piece.rearrange("p (a two c) -> p a two c", two=2, c=32)
                nc.vector.tensor_tensor(
                    out=t1x.rearrange("p (a c) -> p a c", c=32)[
                        :, lo * rows:hi * rows],
                    in0=gv[:, :, 0, :], in1=gv[:, :, 1, :], op=AO.add)

            # ---- half 0: chunks 0-7, quad muls + [4,4] L1 ---------------
            t1 = tpool.tile([BS, 128 * 32], BF16, tag="t1")
            ghalf = gpool.tile([128, 8 * CHUNK], BF16, tag="g")
            for qq in range(8):
                consume(qq, ghalf, qq)
                if qq % 4 == 3:
                    l1p(t1, ghalf, qq - 3, qq + 1)
            t6 = tree(t1, 128)
            kb0 = kbpool.tile([BS, 128], FP32, tag="kb0", name="kb0")
            nc.vector.tensor_tensor(out=kb0[:], in0=t6[:], in1=bc_sb(s)[:, 0:128],
                                    op=AO.add)
            kb_halves.append(kb0)
            if post_half is not None:
                post_half(0, kb0)

            # ---- half 1: chunks 8-15, fine tail pieces ------------------
            t1b = tpool.tile([BS, 128 * 32], BF16, tag="t1")
            ghalf = gpool.tile([128, 8 * CHUNK], BF16, tag="g")
            for qq in range(8):
                consume(8 + qq, ghalf, qq)
                if qq == 3:
                    l1p(t1b, ghalf, 0, 4)
                elif qq == 7:
                    l1p(t1b, ghalf, 4, 8)
            t6b = tree(t1b, 128)
            kb1 = kbpool.tile([BS, 128], FP32, tag="kb1", name="kb1")
            nc.vector.tensor_tensor(out=kb1[:], in0=t6b[:],
                                    in1=bc_sb(s)[:, 128:256], op=AO.add)
            kb_halves.append(kb1)
            if post_half is not None:
                post_half(1, kb1)
            return kb_halves

        # ---- RK4 time loop ----------------------------------------------
        for i in range(T - 1):
            dt_i, im, fm = cs[i]
            hdt = float(f32(f32(0.5) * f32(dt_i)))
            dt6 = float(f32(f32(dt_i) / f32(6.0)))
            s_m = 10 + i
            s_e = (i + 1) if i < T - 2 else 20

            def mk_zs(kb_alpha, tag):
                tiles = [zpool.tile([BS, 128], BF16, tag=f"{tag}{h}", name=f"{tag}{h}") for h in range(2)]

                def post(hh, kbh):
                    nc.vector.scalar_tensor_tensor(
                        out=tiles[hh][:], in0=kbh[:], scalar=kb_alpha,
                        in1=z[hh][:], op0=AO.mult, op1=AO.add)
                return tiles, post

            zs1, post1 = mk_zs(hdt, "zs1_")
            kb1 = gstage(zc, i, 1, emit_out_t=i, post_half=post1)

            zs2, post2 = mk_zs(hdt, "zs2_")
            kb2 = gstage(zs1, s_m, 2, post_half=post2)

            zs3, post3 = mk_zs(float(dt_i), "zs3_")
            kb3 = gstage(zs2, s_m, 3, post_half=post3)

            # partial RK4 combine: lands in DVE's idle window at k4 start
            acc2 = [kbpool.tile([BS, 128], FP32, tag=f"ac2{h}", name=f"ac2{h}") for h in range(2)]
            for h in range(2):
                acc = kbpool.tile([BS, 128], FP32, tag=f"acc{h}", name=f"acc{h}")
                nc.vector.scalar_tensor_tensor(out=acc[:], in0=kb2[h][:], scalar=2.0,
                                               in1=kb1[h][:], op0=AO.mult, op1=AO.add)
                nc.vector.scalar_tensor_tensor(out=acc2[h][:], in0=kb3[h][:], scalar=2.0,
                                               in1=acc[:], op0=AO.mult, op1=AO.add)

            znew = [zpool.tile([BS, 128], FP32, tag=f"z{h}", name=f"zn{h}") for h in range(2)]
            zcn = [zpool.tile([BS, 128], BF16, tag=f"zc{h}", name=f"zcn{h}") for h in range(2)]

            def post4(hh, kbh):
                acc3 = kbpool.tile([BS, 128], FP32, tag=f"ac3{hh}", name=f"ac3{hh}")
                nc.vector.tensor_tensor(out=acc3[:], in0=kbh[:], in1=acc2[hh][:],
                                        op=AO.add)
                nc.vector.scalar_tensor_tensor(out=znew[hh][:], in0=acc3[:], scalar=dt6,
                                               in1=z[hh][:], op0=AO.mult, op1=AO.add)
                nc.vector.tensor_copy(zcn[hh][:], znew[hh][:])

            gstage(zs3, s_e, 4, post_half=post4)
            z, zc = znew, zcn

        # ---- final out row (t = T-1) ------------------------------------
        ot_ps = pp.tile([128, H], FP32, tag="ht")
        for hh in range(2):
            ztp = zt_pool.tile([128, 128], BF16, tag="ztb")
            nc.tensor.transpose(ztp[:], zc[hh][:], identb[:])
            zTbh = spool.tile([128, 128], BF16, tag=f"zTb{hh}", name=f"zTb{hh}")
            nc.scalar.copy(zTbh[:], ztp[:])
            nc.tensor.matmul(ot_ps[:, 0:C], lhsT=zTbh[:],
                             rhs=wout_sb[:, hh * C:(hh + 1) * C],
                             start=(hh == 0), stop=(hh == 1))
        nc.vector.tensor_tensor(out=out_sb[:, (T - 1) * C:T * C],
                                in0=ot_ps[:, 0:C], in1=bout_rep[:], op=AO.add)

        nc.sync.dma_start(out=out_d, in_=out_sb[:])

    nc.compile()
    return nc


def _w2_cmajor(W2):
    """Fold W2 [H, H*C] (c-minor column order) rows into the DoubleRow
    [p, (ktile, n)] rhs layout."""
    wcm = np.asarray(W2, dtype=f32).reshape(H, HC)
    return wcm.reshape(2, 128, HC).transpose(1, 0, 2).reshape(128, 2 * HC)


_CACHE = {}


def _get_program(t_span: np.ndarray, w2_scale: float):
    key = (np.asarray(t_span, dtype=f32).tobytes(), float(w2_scale))
    if key not in _CACHE:
        _CACHE[key] = _build_program(t_span, w2_scale)
    return _CACHE[key]


def _w2_scale(W2):
    return float(240.0 / (1.25 * np.abs(np.asarray(W2, dtype=f32)).max()))


def _make_in_maps(inputs):
    coeffs = np.ascontiguousarray(inputs["coeffs"], dtype=f32)
    assert coeffs.shape == (B, T - 1, 4 * C)
    wcm = _w2_cmajor(inputs["W2"])
    s = _w2_scale(inputs["W2"])
    shared = {
        "w1": np.ascontiguousarray(inputs["W1"], dtype=f32).astype(bf16),
        "b1": np.ascontiguousarray(inputs["b1"], dtype=f32),
        "b2rt": np.ascontiguousarray(
            np.asarray(inputs["b2"], dtype=f32).reshape(H, C).T).astype(bf16),
        "winit": np.ascontiguousarray(inputs["W_init"], dtype=f32).astype(bf16),
        "wout": np.ascontiguousarray(inputs["W_out"], dtype=f32).astype(bf16),
        "binit": np.ascontiguousarray(inputs["b_init"], dtype=f32).reshape(1, H),
        "bout": np.ascontiguousarray(inputs["b_out"], dtype=f32).reshape(1, C),
    }
    if NEED_FP8:
        shared["w2f8"] = np.ascontiguousarray(
            np.clip(wcm * s, -240.0, 240.0)).astype(fp8)
    if NEED_BF16:
        shared["w2b16"] = np.ascontiguousarray(wcm).astype(bf16)
    in_maps = []
    for c in range(NCORES):
        m = dict(shared)
        m["coeffs"] = coeffs[c * BS:(c + 1) * BS]
        in_maps.append(m)
    return in_maps


def kernel(coeffs, t_span, W_init, b_init, W1, b1, W2, b2, W_out, b_out):
    nc = _get_program(t_span, _w2_scale(W2))
    in_maps = _make_in_maps(dict(coeffs=coeffs, W_init=W_init, b_init=b_init,
                                 W1=W1, b1=b1, W2=W2, b2=b2,
                                 W_out=W_out, b_out=b_out))
    res = run_bass_kernel_spmd(nc, in_maps, list(range(NCORES)))
    shards = [res.results[c]["out"].reshape(BS, T, C) for c in range(NCORES)]
    return np.ascontiguousarray(np.concatenate(shards, axis=0), dtype=f32)


if __name__ == "__main__":
    rng = np.random.default_rng(0)
    demo = dict(
        coeffs=(rng.standard_normal((B, T - 1, 4 * C)) * 0.5).astype(f32),
        t_span=(np.arange(T) * 0.05).astype(f32),
        W_init=(rng.standard_normal((C, H)) / 8).astype(f32),
        b_init=(rng.standard_normal((H,)) * 0.01).astype(f32),
        W1=(rng.standard_normal((H, H)) / 16).astype(f32),
        b1=(rng.standard_normal((H,)) * 0.01).astype(f32),
        W2=(rng.standard_normal((H, HC)) / 16).astype(f32),
        b2=(rng.standard_normal((HC,)) * 0.01).astype(f32),
        W_out=(rng.standard_normal((H, C)) / 16).astype(f32),
        b_out=np.zeros((C,), f32),
    )
    out = kernel(**demo)
    print("out", out.shape, out.dtype, float(np.abs(out).max()))


# revision 45
# speedup vs baseline: 1.1170x; 1.0096x over previous
"""Neural CDE (RK4, 10 steps) Trainium2 Bass/Tile kernel — v3.

Data-parallel over batch: B=1024 split as 128 per core across 8 NeuronCores.
Weights replicated; no collectives.

Design (vs v1 baseline at ~1.41-1.68ms, now ~1.18ms):
  * W2 matmul in fp8e4 DoubleRow perf mode: K=256 contracted in a single
    instruction per 256-col window at 0.5 cyc/col — h quantizes for free
    via the tanh ACT output dtype; W2 host-quantized with a global scale
    folded into the per-stage dX multipliers.  rel_err ~1.6e-2 (gate 2e-2).
  * F streams through PSUM in 16 chunks of 1024 (3-deep PSUM ring).
    ACT plain-copies each chunk to SBUF bf16 (1 elem/cyc/lane);
    DVE multiplies by broadcast dX at 2x-packed rate.
  * Segment reduction over c as a 2x-packed bf16 pairwise add-tree on
    DVE (tensor_reduce has no DVE perf mode and runs 1x), batched into
    4-chunk L1 pieces feeding per-h'-half deep levels.
  * The stage is split into h'-halves: half-0's tree, z-update,
    PE transpose and W1 k-chunk all overlap half-1's crossing, keeping
    the serial RK4 stage tail short.
  * b2 term enters as bc = dX @ b2r.T (PE, precomputed per stage) added
    at the tree root; out rows are projected from the bf16 zT halves.
"""

import sys
import numpy as np

for _p in ("/opt/trn_rl_repo",):
    if _p not in sys.path:
        sys.path.insert(0, _p)

import ml_dtypes
from contextlib import ExitStack

import concourse.bass as bass
import concourse.bacc as bacc
import concourse.mybir as mybir
import concourse.tile as tile
from concourse.masks import make_identity
from concourse.bass_utils import run_bass_kernel_spmd

B, T, C, H = 1024, 11, 64, 256
NCORES = 8
BS = B // NCORES          # 128
HC = H * C                # 16384

# ---- tuning knobs ------------------------------------------------------
# dtype per RK4 slot (1..4): "fp8" or "bf16"
SLOT_DT = {1: "fp8", 2: "fp8", 3: "fp8", 4: "fp8"}
N_FUSED_DVE = 0           # chunks consumed by DVE directly from PSUM (fused mul)
N_GPS_MUL = 0             # chunks whose bf16 multiply runs on GpSimd

f32 = np.float32
bf16 = ml_dtypes.bfloat16
fp8 = ml_dtypes.float8_e4m3   # TRN float8e4: max ±240
FP32 = mybir.dt.float32
BF16 = mybir.dt.bfloat16
FP8 = mybir.dt.float8e4
AO = mybir.AluOpType
AF = mybir.ActivationFunctionType
AX = mybir.AxisListType
DR = mybir.MatmulPerfMode.DoubleRow

NEED_FP8 = "fp8" in SLOT_DT.values()
NEED_BF16 = "bf16" in SLOT_DT.values()

CHUNK = 1024                  # PSUM chunk cols (16 h'-rows of 64 c)
NCHUNK = HC // CHUNK          # 16 chunks; chunks [0,8) = h'-half 0


def _stage_consts(t_span: np.ndarray):
    """Host-side f32 scalar constants mimicking the reference's fp32 ops."""
    t = np.asarray(t_span, dtype=f32)
    cs = []
    for i in range(T - 1):
        t0 = t[i]
        dt = f32(t[i + 1] - t0)
        tm = f32(t0 + f32(f32(0.5) * dt))
        idx_m = int(np.clip(np.searchsorted(t, tm, side="right") - 1, 0, T - 2))
        fm = f32(tm - t[idx_m])
        cs.append((float(dt), idx_m, float(fm)))
    fr_last = f32(t[T - 1] - t[T - 2])
    return cs, float(fr_last)


def _build_program(t_span: np.ndarray, w2_scale: float):
    cs, fr_last = _stage_consts(t_span)
    s_inv = float(1.0 / w2_scale)

    nc = bacc.Bacc("TRN2", target_bir_lowering=False, debug=False,
                   enable_asserts=False, num_devices=NCORES)

    coeffs_d = nc.dram_tensor("coeffs", [BS, T - 1, 4 * C], FP32, kind="ExternalInput").ap()
    w1_d = nc.dram_tensor("w1", [H, H], BF16, kind="ExternalInput").ap()
    if NEED_FP8:
        w2f8_d = nc.dram_tensor("w2f8", [128, 2 * HC], FP8, kind="ExternalInput").ap()
    if NEED_BF16:
        w2b16_d = nc.dram_tensor("w2b16", [128, 2 * HC], BF16, kind="ExternalInput").ap()
    b1_d = nc.dram_tensor("b1", [H], FP32, kind="ExternalInput").ap()
    b2rt_d = nc.dram_tensor("b2rt", [C, H], BF16, kind="ExternalInput").ap()
    winit_d = nc.dram_tensor("winit", [C, H], BF16, kind="ExternalInput").ap()
    wout_d = nc.dram_tensor("wout", [H, C], BF16, kind="ExternalInput").ap()
    binit_d = nc.dram_tensor("binit", [1, H], FP32, kind="ExternalInput").ap()
    bout_d = nc.dram_tensor("bout", [1, C], FP32, kind="ExternalInput").ap()
    out_d = nc.dram_tensor("out", [BS, T * C], FP32, kind="ExternalOutput").ap()

    with tile.TileContext(nc) as tc, ExitStack() as ctx:
        const = ctx.enter_context(tc.tile_pool(name="const", bufs=1))
        spool = ctx.enter_context(tc.tile_pool(name="stage", bufs=2))
        zpool = ctx.enter_context(tc.tile_pool(name="z", bufs=3))
        kbpool = ctx.enter_context(tc.tile_pool(name="kb", bufs=5))
        gpool = ctx.enter_context(tc.tile_pool(name="gsb", bufs=3))
        cpool = ctx.enter_context(tc.tile_pool(name="craw", bufs=3))
        tpool = ctx.enter_context(tc.tile_pool(name="tr", bufs=2))
        pp = ctx.enter_context(tc.tile_pool(name="psmm", bufs=1, space="PSUM"))
        zt_pool = ctx.enter_context(tc.tile_pool(name="pszt", bufs=1, space="PSUM"))
        fp = ctx.enter_context(tc.tile_pool(name="psfp", bufs=3, space="PSUM"))

        # ---- resident tensors -------------------------------------------
        coeffs_sb = const.tile([BS, (T - 1) * 4 * C], FP32, tag="coeffs")
        w1_sb = const.tile([128, 2 * H], BF16, tag="w1")
        if NEED_FP8:
            w2f8_sb = const.tile([128, 2 * HC], FP8, tag="w2f8")
        if NEED_BF16:
            w2b16_sb = const.tile([128, 2 * HC], BF16, tag="w2b16")
        b1_sb = const.tile([128, 2], FP32, tag="b1")
        b2rt_sb = const.tile([C, H], BF16, tag="b2rt")
        winit_sb = const.tile([C, H], BF16, tag="winit")
        wout_sb = const.tile([128, 2 * C], BF16, tag="wout")
        binit_sb = const.tile([1, H], FP32, tag="binit")
        bout_sb = const.tile([1, C], FP32, tag="bout")
        ones1_sb = const.tile([1, 128], FP32, tag="ones1")
        ident = const.tile([128, 128], FP32, tag="ident")
        binit_rep = const.tile([128, H], FP32, tag="binit_rep")
        bout_rep = const.tile([128, C], FP32, tag="bout_rep")
        dxm_sb = const.tile([128, 11 * C], FP32, tag="dxm")      # 10 mids + last-end
        dxT_sb = const.tile([C, 21 * 128], BF16, tag="dxT")
        # per-stage DVE broadcast multipliers (fp8 variant folds 1/s)
        dxb8_sb = const.tile([128, 21 * C], BF16, tag="dxb8")
        dxb16_sb = const.tile([128, 21 * C], BF16, tag="dxb16")
        bc_all = const.tile([128, 21 * H], BF16, tag="bc_all")   # dX @ b2r.T per stage
        out_sb = const.tile([BS, T * C], FP32, tag="out_sb")

        nc.sync.dma_start(out=coeffs_sb[:], in_=coeffs_d.rearrange("p i j -> p (i j)"))
        nc.sync.dma_start(out=w1_sb.rearrange("p (k h) -> p k h", k=2),
                          in_=w1_d.rearrange("(k p) h -> p k h", p=128))
        if NEED_FP8:
            nc.sync.dma_start(out=w2f8_sb[:], in_=w2f8_d)
        if NEED_BF16:
            nc.sync.dma_start(out=w2b16_sb[:], in_=w2b16_d)
        nc.sync.dma_start(out=b1_sb[:], in_=b1_d.rearrange("(k p) -> p k", p=128))
        nc.sync.dma_start(out=b2rt_sb[:], in_=b2rt_d)
        nc.sync.dma_start(out=winit_sb[:], in_=winit_d)
        nc.sync.dma_start(out=wout_sb.rearrange("p (k c) -> p k c", k=2),
                          in_=wout_d.rearrange("(k p) c -> p k c", p=128))
        nc.sync.dma_start(out=binit_sb[:], in_=binit_d)
        nc.sync.dma_start(out=bout_sb[:], in_=bout_d)

        nc.vector.memset(ones1_sb[:], 1.0)
        make_identity(nc, ident[:])
        identb = const.tile([128, 128], BF16, tag="identb")
        nc.vector.tensor_copy(identb[:], ident[:])

        def cview(i, part):
            """f32 view of coeff column `part` (0=a,1=b,2=2c,3=3d) of interval i."""
            off = i * 4 * C + part * C
            return coeffs_sb[:, off:off + C]

        def dx_f32(s):
            if s < 10:
                return cview(s, 1)
            return dxm_sb[:, (s - 10) * C:(s - 9) * C]

        # ---- dX mid/end vectors (f32) -----------------------------------
        tmp_pool = ctx.enter_context(tc.tile_pool(name="tmp64", bufs=2))
        for i in range(T - 1):
            dt_i, im, fm = cs[i]
            tmp = tmp_pool.tile([128, C], FP32, tag="t64")
            nc.vector.scalar_tensor_tensor(
                out=tmp[:], in0=cview(im, 3), scalar=float(fm), in1=cview(im, 2),
                op0=AO.mult, op1=AO.add)
            nc.vector.scalar_tensor_tensor(
                out=dxm_sb[:, i * C:(i + 1) * C], in0=tmp[:], scalar=float(fm),
                in1=cview(im, 1), op0=AO.mult, op1=AO.add)
        tmp = tmp_pool.tile([128, C], FP32, tag="t64")
        nc.vector.scalar_tensor_tensor(
            out=tmp[:], in0=cview(T - 2, 3), scalar=float(fr_last), in1=cview(T - 2, 2),
            op0=AO.mult, op1=AO.add)
        nc.vector.scalar_tensor_tensor(
            out=dxm_sb[:, 10 * C:11 * C], in0=tmp[:], scalar=float(fr_last),
            in1=cview(T - 2, 1), op0=AO.mult, op1=AO.add)

        # ---- per-stage scaled dX variants --------------------------------
        for s in range(21):
            src = dx_f32(s)
            nc.scalar.mul(dxb8_sb[:, s * C:(s + 1) * C], src, float(s_inv))
            nc.scalar.copy(dxb16_sb[:, s * C:(s + 1) * C], src)

        # ---- dX transposes (bf16) + bc = dX @ b2r.T ----------------------
        for s in range(21):
            src = dx_f32(s)
            ps = pp.tile([128, H], FP32, tag="ht")
            nc.tensor.transpose(ps[0:C, 0:128], src, ident[:])
            nc.scalar.copy(dxT_sb[:, s * 128:(s + 1) * 128], ps[0:C, 0:128])
        for s in range(21):
            bc_ps = pp.tile([128, H], FP32, tag="ht")
            nc.tensor.matmul(bc_ps[:], lhsT=dxT_sb[:, s * 128:(s + 1) * 128],
                             rhs=b2rt_sb[:], start=True, stop=True)
            nc.scalar.copy(bc_all[:, s * H:(s + 1) * H], bc_ps[:])

        def bc_sb(s):
            return bc_all[:, s * H:(s + 1) * H]

        # ---- replicated biases -------------------------------------------
        ps = pp.tile([128, H], FP32, tag="ht")
        nc.tensor.matmul(ps[:, 0:H], lhsT=ones1_sb[:], rhs=binit_sb[:], start=True, stop=True)
        nc.scalar.copy(binit_rep[:], ps[:, 0:H])
        ps = pp.tile([128, H], FP32, tag="ht")
        nc.tensor.matmul(ps[:, 0:C], lhsT=ones1_sb[:], rhs=bout_sb[:], start=True, stop=True)
        nc.scalar.copy(bout_rep[:], ps[:, 0:C])

        # ---- z0 (as half pairs: fp32 z + bf16 cast zc) -------------------
        ps = pp.tile([128, H], FP32, tag="ht")
        nc.tensor.transpose(ps[0:C, 0:128], cview(0, 0), ident[:])
        x0T_sb = spool.tile([C, 128], BF16, tag="x0T")
        nc.scalar.copy(x0T_sb[:], ps[0:C, 0:128])
        ps = pp.tile([128, H], FP32, tag="ht")
        nc.tensor.matmul(ps[:, 0:H], lhsT=x0T_sb[:], rhs=winit_sb[:], start=True, stop=True)
        z = [zpool.tile([BS, 128], FP32, tag=f"z{h}", name=f"z{h}") for h in range(2)]
        zc = [zpool.tile([BS, 128], BF16, tag=f"zc{h}", name=f"zc{h}") for h in range(2)]
        for h in range(2):
            nc.vector.tensor_tensor(out=z[h][:], in0=ps[:, h * 128:(h + 1) * 128],
                                    in1=binit_rep[:, h * 128:(h + 1) * 128], op=AO.add)
            nc.vector.tensor_copy(zc[h][:], z[h][:])

        # ---- one RK4 stage ----------------------------------------------
        # zin is a pair of [128,128] half tiles; returns kb = pair of
        # [128,128] fp32 half tiles.
        def gstage(zin, s, slot, alpha=None, zbase=None, zs_out=None,
                   emit_out_t=None, post_half=None):
            dt8 = SLOT_DT[slot] == "fp8"
            # per-half: transpose zin-half (bf16) -> zT-half -> W1 k-chunk
            ht_ps = pp.tile([128, H], FP32, tag="ht")
            if alpha is not None:
                zb = [kbpool.tile([BS, 128], FP32, tag=f"zb{h}", name=f"zb{h}")
                      for h in range(2)]
                for h in range(2):
                    nc.vector.scalar_tensor_tensor(
                        out=zb[h][:], in0=bc_sb(s)[:, h * 128:(h + 1) * 128],
                        scalar=float(alpha), in1=zbase[h][:],
                        op0=AO.mult, op1=AO.add)
            ot_ps = (fp.tile([128, CHUNK], FP32, tag="fp", name="ot_ps")
                     if emit_out_t is not None else None)
            ztpair = zt_pool.tile([128, 256], BF16, tag="ztb", name="ztpair")
            for hh in range(2):
                nc.tensor.transpose(ztpair[:, hh * 128:(hh + 1) * 128],
                                    zin[hh][:], identb[:])
            zTb = spool.tile([128, 256], BF16, tag="zTb", name="zTb")
            nc.scalar.copy(zTb[:], ztpair[:])
            zTbs = [zTb[:, 0:128], zTb[:, 128:256]]
            for hck in range(2):
                for hh in range(2):
                    nc.tensor.matmul(
                        ht_ps[:, hck * 128:(hck + 1) * 128],
                        lhsT=w1_sb[:, hh * H + hck * 128: hh * H + (hck + 1) * 128],
                        rhs=zTbs[hh],
                        start=(hh == 0), stop=(hh == 1))
            if ot_ps is not None:
                for hh in range(2):
                    nc.tensor.matmul(ot_ps[:, 0:C], lhsT=zTbs[hh],
                                     rhs=wout_sb[:, hh * C:(hh + 1) * C],
                                     start=(hh == 0), stop=(hh == 1))
                nc.vector.tensor_tensor(out=out_sb[:, emit_out_t * C:(emit_out_t + 1) * C],
                                        in0=ot_ps[:, 0:C], in1=bout_rep[:], op=AO.add)

            hT = spool.tile([128, H], FP8 if dt8 else BF16,
                            tag="hT8" if dt8 else "hT16", name="hT")
            for hck in range(2):
                nc.scalar.activation(hT[:, hck * 128:(hck + 1) * 128],
                                     ht_ps[:, hck * 128:(hck + 1) * 128],
                                     AF.Tanh, bias=b1_sb[:, hck:hck + 1], scale=1.0)
            hT2 = hT.rearrange("p (k m) -> p k m", k=2)

            if NEED_FP8 and dt8:
                w2v = w2f8_sb.rearrange("p (k n) -> p k n", k=2)
            else:
                w2v = w2b16_sb.rearrange("p (k n) -> p k n", k=2)
            dxb = (dxb8_sb if dt8 else dxb16_sb)[:, s * C:(s + 1) * C]
            dxbc = dxb.rearrange("p (r c) -> p r c", r=1)

            def do_mm(fps, col0, ncols):
                if dt8:
                    for j in range(ncols // 256):
                        nc.tensor.matmul(
                            fps[:, j * 256:(j + 1) * 256],
                            lhsT=hT2, rhs=w2v[:, :, col0 + j * 256: col0 + (j + 1) * 256],
                            start=True, stop=True, perf_mode=DR,
                            skip_group_check=True)
                else:
                    for w in range(ncols // 512):
                        for kc in range(2):
                            nc.tensor.matmul(
                                fps[:, w * 512:(w + 1) * 512],
                                lhsT=hT[:, kc * 128:(kc + 1) * 128],
                                rhs=w2v[:, kc, col0 + w * 512: col0 + (w + 1) * 512],
                                start=(kc == 0), stop=(kc == 1),
                                skip_group_check=True)

            # fused chunks: DVE multiplies straight from PSUM (skips ACT);
            # placed at the end of each half where the ACT queue would
            # otherwise gate the reduce tail
            fused = set()
            gps_mul = set()
            if N_GPS_MUL:
                step = max(1, (NCHUNK - N_FUSED_DVE) // N_GPS_MUL)
                gps_mul = set(range(1, NCHUNK - N_FUSED_DVE, step))

            rows = CHUNK // C              # 16 h'-rows per chunk

            def consume(q, ghalf, qq, solo=False):
                # PE chunk -> ACT copy (own ring tile) -> DVE 2x multiply
                fps = fp.tile([128, CHUNK], FP32, tag="fp", name="fps")
                do_mm(fps, q * CHUNK, CHUNK)
                craw = cpool.tile([128, CHUNK], BF16, tag="craw", name="craw")
                nc.scalar.copy(craw[:], fps[:])
                nc.vector.tensor_tensor(
                    out=ghalf[:, qq * CHUNK:(qq + 1) * CHUNK].rearrange(
                        "p (a c) -> p a c", c=C),
                    in0=craw.rearrange("p (a c) -> p a c", c=C),
                    in1=dxbc.broadcast_to([128, rows, C]), op=AO.mult)

            def tree(t1, nrows):
                # [p, nrows, 32] -> [p, nrows] via halving adds
                cur, width = t1, 32
                while width > 2:
                    nw = width // 2
                    vv = cur.rearrange("p (a two c) -> p a two c", two=2, c=nw)
                    dst = tpool.tile([BS, nrows * nw], BF16, tag=f"tl{nw}",
                                     name=f"tl{nw}")
                    nc.vector.tensor_tensor(
                        out=dst.rearrange("p (a c) -> p a c", c=nw),
                        in0=vv[:, :, 0, :], in1=vv[:, :, 1, :], op=AO.add)
                    cur, width = dst, nw
                vv = cur.rearrange("p (a two) -> p a two", two=2)
                t6 = tpool.tile([BS, nrows], BF16, tag="t6", name="t6")
                nc.vector.tensor_tensor(out=t6.rearrange("p (a o) -> p a o", o=1),
                                        in0=vv[:, :, 0:1], in1=vv[:, :, 1:2],
                                        op=AO.add)
                return t6

            kb_halves = []

            def l1p(t1x, ghalf, lo, hi):
                piece = ghalf[:, lo * CHUNK:hi * CHUNK]
                gv = piece.rearrange("p (a two c) -> p a two c", two=2, c=32)
                nc.vector.tensor_tensor(
                    out=t1x.rearrange("p (a c) -> p a c", c=32)[
                        :, lo * rows:hi * rows],
                    in0=gv[:, :, 0, :], in1=gv[:, :, 1, :], op=AO.add)

            # ---- half 0: chunks 0-7, quad muls + [4,4] L1 ---------------
            t1 = tpool.tile([BS, 128 * 32], BF16, tag="t1")
            ghalf = gpool.tile([128, 8 * CHUNK], BF16, tag="g")
            for qq in range(8):
                consume(qq, ghalf, qq)
                if qq % 4 == 3:
                    l1p(t1, ghalf, qq - 3, qq + 1)
            t6 = tree(t1, 128)
            if alpha is not None:
                nc.vector.scalar_tensor_tensor(
                    out=zs_out[0][:], in0=t6[:], scalar=float(alpha),
                    in1=zb[0][:], op0=AO.mult, op1=AO.add)
            if slot != 4:
                kb0 = kbpool.tile([BS, 128], FP32, tag="kb0", name="kb0")
                nc.vector.tensor_tensor(out=kb0[:], in0=t6[:],
                                        in1=bc_sb(s)[:, 0:128], op=AO.add)
                kb_halves.append(kb0)
            if post_half is not None:
                post_half(0, t6)

            # ---- half 1: chunks 8-15, fine tail pieces ------------------
            t1b = tpool.tile([BS, 128 * 32], BF16, tag="t1")
            ghalf = gpool.tile([128, 8 * CHUNK], BF16, tag="g")
            for qq in range(8):
                consume(8 + qq, ghalf, qq)
                if qq == 3:
                    l1p(t1b, ghalf, 0, 4)
                elif qq == 7:
                    l1p(t1b, ghalf, 4, 8)
            t6b = tree(t1b, 128)
            if alpha is not None:
                nc.vector.scalar_tensor_tensor(
                    out=zs_out[1][:], in0=t6b[:], scalar=float(alpha),
                    in1=zb[1][:], op0=AO.mult, op1=AO.add)
            if slot != 4:
                kb1 = kbpool.tile([BS, 128], FP32, tag="kb1", name="kb1")
                nc.vector.tensor_tensor(out=kb1[:], in0=t6b[:],
                                        in1=bc_sb(s)[:, 128:256], op=AO.add)
                kb_halves.append(kb1)
            if post_half is not None:
                post_half(1, t6b)
            return kb_halves

        # ---- RK4 time loop ----------------------------------------------
        for i in range(T - 1):
            dt_i, im, fm = cs[i]
            hdt = float(f32(f32(0.5) * f32(dt_i)))
            dt6 = float(f32(f32(dt_i) / f32(6.0)))
            s_m = 10 + i
            s_e = (i + 1) if i < T - 2 else 20

            def mk_zs(tag):
                return [zpool.tile([BS, 128], BF16, tag=f"{tag}{h}",
                                   name=f"{tag}{h}") for h in range(2)]

            zs1 = mk_zs("zs1_")
            kb1 = gstage(zc, i, 1, alpha=hdt, zbase=z, zs_out=zs1, emit_out_t=i)

            zs2 = mk_zs("zs2_")
            kb2 = gstage(zs1, s_m, 2, alpha=hdt, zbase=z, zs_out=zs2)

            zs3 = mk_zs("zs3_")
            kb3 = gstage(zs2, s_m, 3, alpha=float(dt_i), zbase=z, zs_out=zs3)

            # partial RK4 combine: lands in DVE's idle window at k4 start;
            # k4's bc folds into acc2 so its tree root feeds acc3 directly
            acc2b = [kbpool.tile([BS, 128], FP32, tag=f"a2b{h}", name=f"a2b{h}") for h in range(2)]
            for h in range(2):
                acc = kbpool.tile([BS, 128], FP32, tag=f"acc{h}", name=f"acc{h}")
                nc.vector.scalar_tensor_tensor(out=acc[:], in0=kb2[h][:], scalar=2.0,
                                               in1=kb1[h][:], op0=AO.mult, op1=AO.add)
                acc2 = kbpool.tile([BS, 128], FP32, tag=f"ac2{h}", name=f"ac2{h}")
                nc.vector.scalar_tensor_tensor(out=acc2[:], in0=kb3[h][:], scalar=2.0,
                                               in1=acc[:], op0=AO.mult, op1=AO.add)
                nc.vector.tensor_tensor(out=acc2b[h][:], in0=acc2[:],
                                        in1=bc_sb(s_e)[:, h * 128:(h + 1) * 128],
                                        op=AO.add)

            znew = [zpool.tile([BS, 128], FP32, tag=f"z{h}", name=f"zn{h}") for h in range(2)]
            zcn = [zpool.tile([BS, 128], BF16, tag=f"zc{h}", name=f"zcn{h}") for h in range(2)]

            def post4(hh, t6h):
                acc3 = kbpool.tile([BS, 128], FP32, tag=f"ac3{hh}", name=f"ac3{hh}")
                nc.vector.tensor_tensor(out=acc3[:], in0=t6h[:], in1=acc2b[hh][:],
                                        op=AO.add)
                nc.vector.scalar_tensor_tensor(out=znew[hh][:], in0=acc3[:], scalar=dt6,
                                               in1=z[hh][:], op0=AO.mult, op1=AO.add)
                nc.vector.tensor_copy(zcn[hh][:], znew[hh][:])

            gstage(zs3, s_e, 4, post_half=post4)
            z, zc = znew, zcn

        # ---- final out row (t = T-1) ------------------------------------
        ot_ps = pp.tile([128, H], FP32, tag="ht")
        for hh in range(2):
            ztp = zt_pool.tile([128, 128], BF16, tag="ztb")
            nc.tensor.transpose(ztp[:], zc[hh][:], identb[:])
            zTbh = spool.tile([128, 128], BF16, tag=f"zTb{hh}", name=f"zTb{hh}")
            nc.scalar.copy(zTbh[:], ztp[:])
            nc.tensor.matmul(ot_ps[:, 0:C], lhsT=zTbh[:],
                             rhs=wout_sb[:, hh * C:(hh + 1) * C],
                             start=(hh == 0), stop=(hh == 1))
        nc.vector.tensor_tensor(out=out_sb[:, (T - 1) * C:T * C],
                                in0=ot_ps[:, 0:C], in1=bout_rep[:], op=AO.add)

        nc.sync.dma_start(out=out_d, in_=out_sb[:])

    nc.compile()
    return nc


def _w2_cmajor(W2):
    """Fold W2 [H, H*C] (c-minor column order) rows into the DoubleRow
    [p, (ktile, n)] rhs layout."""
    wcm = np.asarray(W2, dtype=f32).reshape(H, HC)
    return wcm.reshape(2, 128, HC).transpose(1, 0, 2).reshape(128, 2 * HC)


_CACHE = {}


def _get_program(t_span: np.ndarray, w2_scale: float):
    key = (np.asarray(t_span, dtype=f32).tobytes(), float(w2_scale))
    if key not in _CACHE:
        _CACHE[key] = _build_program(t_span, w2_scale)
    return _CACHE[key]


def _w2_scale(W2):
    return float(240.0 / (1.25 * np.abs(np.asarray(W2, dtype=f32)).max()))


def _make_in_maps(inputs):
    coeffs = np.ascontiguousarray(inputs["coeffs"], dtype=f32)
    assert coeffs.shape == (B, T - 1, 4 * C)
    wcm = _w2_cmajor(inputs["W2"])
    s = _w2_scale(inputs["W2"])
    shared = {
        "w1": np.ascontiguousarray(inputs["W1"], dtype=f32).astype(bf16),
        "b1": np.ascontiguousarray(inputs["b1"], dtype=f32),
        "b2rt": np.ascontiguousarray(
            np.asarray(inputs["b2"], dtype=f32).reshape(H, C).T).astype(bf16),
        "winit": np.ascontiguousarray(inputs["W_init"], dtype=f32).astype(bf16),
        "wout": np.ascontiguousarray(inputs["W_out"], dtype=f32).astype(bf16),
        "binit": np.ascontiguousarray(inputs["b_init"], dtype=f32).reshape(1, H),
        "bout": np.ascontiguousarray(inputs["b_out"], dtype=f32).reshape(1, C),
    }
    if NEED_FP8:
        shared["w2f8"] = np.ascontiguousarray(
            np.clip(wcm * s, -240.0, 240.0)).astype(fp8)
    if NEED_BF16:
        shared["w2b16"] = np.ascontiguousarray(wcm).astype(bf16)
    in_maps = []
    for c in range(NCORES):
        m = dict(shared)
        m["coeffs"] = coeffs[c * BS:(c + 1) * BS]
        in_maps.append(m)
    return in_maps


def kernel(coeffs, t_span, W_init, b_init, W1, b1, W2, b2, W_out, b_out):
    nc = _get_program(t_span, _w2_scale(W2))
    in_maps = _make_in_maps(dict(coeffs=coeffs, W_init=W_init, b_init=b_init,
                                 W1=W1, b1=b1, W2=W2, b2=b2,
                                 W_out=W_out, b_out=b_out))
    res = run_bass_kernel_spmd(nc, in_maps, list(range(NCORES)))
    shards = [res.results[c]["out"].reshape(BS, T, C) for c in range(NCORES)]
    return np.ascontiguousarray(np.concatenate(shards, axis=0), dtype=f32)


if __name__ == "__main__":
    rng = np.random.default_rng(0)
    demo = dict(
        coeffs=(rng.standard_normal((B, T - 1, 4 * C)) * 0.5).astype(f32),
        t_span=(np.arange(T) * 0.05).astype(f32),
        W_init=(rng.standard_normal((C, H)) / 8).astype(f32),
        b_init=(rng.standard_normal((H,)) * 0.01).astype(f32),
        W1=(rng.standard_normal((H, H)) / 16).astype(f32),
        b1=(rng.standard_normal((H,)) * 0.01).astype(f32),
        W2=(rng.standard_normal((H, HC)) / 16).astype(f32),
        b2=(rng.standard_normal((HC,)) * 0.01).astype(f32),
        W_out=(rng.standard_normal((H, C)) / 16).astype(f32),
        b_out=np.zeros((C,), f32),
    )
    out = kernel(**demo)
    print("out", out.shape, out.dtype, float(np.abs(out).max()))
